# revision 1
# baseline (speedup 1.0000x reference)
"""Multi-head attention (B=2, H=16, S=4096, D=64, fp16) on 8 TRN2 NeuronCores.

Sharding: the 32 (b, h) head-slices are split 4-per-core (data/head
parallel, no cross-core communication). Each core runs a flash-attention
style kernel over its 4 heads.

Per-head algorithm (transposed-scores layout, no on-device transposes in
the hot loop):
  - Host pre-lays-out inputs: QT[d, s] = Q^T, KTp[d, j*128+p] = K[p*32+j, d]
    (a t-permutation that makes the V load contiguous), and VA = [V | 1]
    (ones column => the PV matmul also accumulates the softmax normalizer).
    QT/KT are loaded twice (partitions 0-63 and 64-127) so score matmuls can
    be row-packed onto both halves of the PE array (concurrent execution,
    weight loads pull ahead).
  - scores^T tile [t=128, s=512] = KTp_tile.T @ QT_tile   (PE, K=64)
  - P^T = exp(scale * scores^T)  fp32->fp16                (ACT, reads PSUM,
    1536-wide activations over 3 PSUM banks)
  - out^T [65, s] += VA_tile.T @ P^T_tile                  (PE, K=128; each
    VA stationary is loaded once and reused for the two 512-chunks of an
    s-window — the second matmul is marked non-self-loading)
    row 64 of out^T = sum_t P^T[t, s] = softmax denominator.
  - fixup per 1024-wide s-window: copy PSUM->SBUF, PE-transpose to
    [s=128, 65] blocks, reciprocal of col 64, per-partition scalar multiply,
    DMA out [s, d].

The emission runs a one-window software pipeline: while window w's scores
stream through PE->ACT, the PV matmuls consume window w-1's probs (already
in SBUF). That keeps ACT (the bottleneck engine) continuously fed and makes
both matmuls of each weight-sharing PV pair schedulable back-to-back.
`verify_ldw_pairs` checks the final PE order for every non-self-loading
matmul at build time.

Softmax skips max-subtraction: scores ~ N(0,1) after scaling, so fp32
exp/sum are numerically safe (|score*scale| < ~7 << 88).
"""

from contextlib import ExitStack

import numpy as np

import concourse.bass as bass
import concourse.tile as tile
from concourse import bacc, mybir
from concourse.bass_utils import run_bass_kernel_spmd
from concourse.masks import make_identity

B, H, S, D = 2, 16, 4096, 64
N_CORES = 8
HPC = (B * H) // N_CORES  # heads per core
SCALE = float(D) ** -0.5
SQ = 512  # s-chunk width (one PSUM bank of fp32)
G = 3  # t-tiles (PSUM banks) per exp group
WIN = 2 * SQ  # s-window: two chunks share each loaded PV stationary

ROWPACK_SCORES = True  # tile_position row-packed scores matmuls
PV_LDW_DEDUP = True  # share one weight load across each PV chunk pair
WARMUP = True  # HAM warmup matmul block


def attention_body(tc, qt, kt, va, o, heads, s, d):
    """Emit the per-core attention program.

    qt: [heads, d, s] fp16   Q^T per head
    kt: [heads, d, s] fp16   K^T per head, t-permuted (col j*128+p = row p*(s//128)+j)
    va: [heads, s, d+1] fp16 V with ones column
    o:  [heads, s, d] fp16   output
    """
    nc = tc.nc
    f32 = mybir.dt.float32
    f16 = mybir.dt.float16
    nt = s // 128  # number of 128-row t tiles
    nwin = s // WIN  # s windows per head
    nq = WIN // 128  # output row blocks per window

    groups = []
    t0 = 0
    while t0 < nt:
        gs = min(G, nt - t0)
        groups.append((t0, gs))
        t0 += gs

    with ExitStack() as ctx:
        qk_pool = ctx.enter_context(tc.tile_pool(name="qk", bufs=2))
        v_pool = ctx.enter_context(tc.tile_pool(name="v", bufs=2))
        # probs live from their exp (window w) until consumed by PV during
        # window w+1: ~2 windows of groups in flight.
        p_pool = ctx.enter_context(
            tc.tile_pool(name="p", bufs=2 * len(groups) + 2)
        )
        ps_pool = ctx.enter_context(tc.tile_pool(name="ps", bufs=2, space="PSUM"))
        po_pool = ctx.enter_context(tc.tile_pool(name="po", bufs=2, space="PSUM"))
        fix_pool = ctx.enter_context(tc.tile_pool(name="fix", bufs=2))
        const_pool = ctx.enter_context(tc.tile_pool(name="const", bufs=1))

        ident = const_pool.tile([d + 1, d + 1], f32)
        make_identity(nc, ident)

        if WARMUP:
            # ~16 back-to-back matmuls trip the HAM activity window early so
            # the PE runs at 2.4 GHz instead of staying clock-gated at 1.2.
            warm_src = const_pool.tile([d + 1, SQ], f16)
            nc.vector.memset(warm_src, 1.0)
            warm_w = const_pool.tile([d + 1, d + 1], f16)
            nc.vector.memset(warm_w, 1.0)
            warm_ps = ps_pool.tile([128, G, SQ], f32, tag="ps")
            for i in range(16):
                nc.tensor.matmul(
                    warm_ps[: d + 1, 0, :],
                    lhsT=warm_w,
                    rhs=warm_src,
                    start=True,
                    stop=True,
                )

        # Per-head SBUF tiles, fetched lazily at head boundaries.
        head_tiles = {}

        def load_head(h):
            # Chunked loads ordered by first use so the first window's scores
            # only wait on the leading slices (Tile tracks byte-range deps).
            nck = 4
            cs = s // nck
            qt_sb = qk_pool.tile([128 if ROWPACK_SCORES else 64, s], f16, tag="qt")
            kt_sb = qk_pool.tile([128 if ROWPACK_SCORES else 64, s], f16, tag="kt")
            va_sb = v_pool.tile([128, nt, d + 1], f16, tag="va")
            va_src = va[h].rearrange("(p i) e -> p i e", p=128)
            rows = [0, 64] if ROWPACK_SCORES else [0]
            ick = nt // nck

            def kt_chunk(k):
                sl = slice(k * cs, (k + 1) * cs)
                for rp in rows:
                    nc.sync.dma_start(out=kt_sb[rp : rp + 64, sl], in_=kt[h][:, sl])

            def qt_chunk(k):
                sl = slice(k * cs, (k + 1) * cs)
                for rp in rows:
                    nc.sync.dma_start(out=qt_sb[rp : rp + 64, sl], in_=qt[h][:, sl])

            # kt chunk 0 + qt chunk 0 unblock the first window's scores; va is
            # first needed a window later; qt tails are needed last.
            kt_chunk(0)
            qt_chunk(0)
            for k in range(1, nck):
                kt_chunk(k)
            for k in range(nck):
                nc.sync.dma_start(
                    out=va_sb[:, k * ick : (k + 1) * ick, :],
                    in_=va_src[:, k * ick : (k + 1) * ick, :],
                )
            for k in range(1, nck):
                qt_chunk(k)
            head_tiles[h] = (qt_sb, kt_sb, va_sb)

        def emit_scores(h, w):
            """Scores + exp for window w of head h; returns per-group pt tiles."""
            qt_sb, kt_sb, _ = head_tiles[h]
            w0 = w * WIN
            win_pts = []
            for t0, gs in groups:
                pts = []
                for c in (0, 1):
                    ps = ps_pool.tile([128, G, SQ], f32, tag="ps")
                    for g in range(gs):
                        t = t0 + g
                        rp = 64 * (t % 2) if ROWPACK_SCORES else 0
                        nc.tensor.matmul(
                            ps[:, g, :],
                            lhsT=kt_sb[rp : rp + 64, t * 128 : (t + 1) * 128],
                            rhs=qt_sb[
                                rp : rp + 64, w0 + c * SQ : w0 + (c + 1) * SQ
                            ],
                            start=True,
                            stop=True,
                            tile_position=(rp, 0) if ROWPACK_SCORES else None,
                        )
                    pt = p_pool.tile([128, G, SQ], f16, tag="pt")
                    nc.scalar.activation(
                        pt[:, :gs, :],
                        ps[:, :gs, :],
                        mybir.ActivationFunctionType.Exp,
                        scale=SCALE,
                    )
                    pts.append(pt)
                win_pts.append(pts)
            return win_pts

        def emit_pv_fixup(h, w, win_pts):
            """PV accumulation + normalize/store for window w of head h."""
            _, _, va_sb = head_tiles[h]
            w0 = w * WIN
            nqc = SQ // 128  # output row blocks per chunk
            pos = [
                po_pool.tile([d + 1, SQ], f32, tag="po", name=f"po{c}_{h}_{w}")
                for c in (0, 1)
            ]
            for (t0, gs), pts in zip(groups, win_pts):
                for g in range(gs):
                    t = t0 + g
                    first = t == 0
                    last = t == nt - 1
                    for c in (0, 1):
                        nc.tensor.matmul(
                            pos[c],
                            lhsT=va_sb[:, t, :],
                            rhs=pts[c][:, g, :],
                            start=first,
                            stop=last,
                        )

            # Per-chunk fixup chains so each PSUM bank frees as early as
            # possible (the po pool slot gates the next window's PV).
            o16 = fix_pool.tile([128, nq, d], f16, tag="o16")
            for c in (0, 1):
                osb = fix_pool.tile([d + 1, SQ], f32, tag=f"osb{c}")
                nc.vector.tensor_copy(osb, pos[c])
                pt4 = po_pool.tile([128, nqc, 128], f32, tag="po")
                for qq in range(nqc):
                    nc.tensor.transpose(
                        pt4[:, qq, 0 : d + 1],
                        osb[:, qq * 128 : (qq + 1) * 128],
                        ident,
                    )
                rec = fix_pool.tile([128, nqc], f32, tag=f"rec{c}")
                nc.vector.reciprocal(rec, pt4[:, :, d])
                nc.vector.tensor_tensor(
                    out=o16[:, c * nqc : (c + 1) * nqc, :],
                    in0=pt4[:, :, 0:d],
                    in1=rec.unsqueeze(2).broadcast_to([128, nqc, d]),
                    op=mybir.AluOpType.mult,
                )
            nc.sync.dma_start(
                out=o[h, w0 : w0 + WIN, :].rearrange("(q p) d -> p q d", p=128),
                in_=o16,
            )

        windows = [(h, w) for h in range(heads) for w in range(nwin)]
        prev = None  # (h, w, win_pts) pending PV
        for i, (h, w) in enumerate(windows):
            if w == 0:
                load_head(h)
            win_pts = emit_scores(h, w)
            if prev is not None:
                emit_pv_fixup(*prev)
            prev = (h, w, win_pts)
        emit_pv_fixup(*prev)


def strip_redundant_ldweights(nc, strip=True):
    """Tile legalization emits one InstLdweights before every (non-transpose)
    matmul. When consecutive loads target identical weights and the later one
    carries no semaphore traffic, drop it — the PE array still holds those
    weights. Transpose matmuls self-load their input into the array, so they
    reset the tracked state. The same walk verifies that every matmul's
    stationary operand matches the weights actually resident."""
    removed = 0
    for f in nc.m.functions:
        for bb in f.blocks:
            insts = list(bb.instructions)
            keep = []
            last_w = None
            changed = False
            for ins in insts:
                if isinstance(ins, mybir.InstLdweights):
                    w = str(ins.ins[0])
                    if (
                        strip
                        and w == last_w
                        and not ins.has_wait()
                        and not ins.has_update()
                    ):
                        removed += 1
                        changed = True
                        continue
                    last_w = w
                elif isinstance(ins, mybir.InstMatmult):
                    if ins.is_transpose:
                        last_w = None  # transpose loads its input into the array
                    else:
                        w = str(ins.ins[1])
                        assert last_w == w, (
                            f"{ins.name}: stationary mismatch\n"
                            f"loaded: {last_w}\nneeds:  {w}"
                        )
                keep.append(ins)
            if changed:
                bb.instructions = keep
    return removed


def build_program(heads=HPC, s=S, d=D):
    nc = bacc.Bacc(
        "TRN2", target_bir_lowering=False, debug=False, num_devices=N_CORES
    )
    qt = nc.dram_tensor("qt", [heads, d, s], mybir.dt.float16, kind="ExternalInput").ap()
    kt = nc.dram_tensor("kt", [heads, d, s], mybir.dt.float16, kind="ExternalInput").ap()
    va = nc.dram_tensor(
        "va", [heads, s, d + 1], mybir.dt.float16, kind="ExternalInput"
    ).ap()
    o = nc.dram_tensor("o", [heads, s, d], mybir.dt.float16, kind="ExternalOutput").ap()
    with tile.TileContext(nc) as tc:
        attention_body(tc, qt, kt, va, o, heads, s, d)
    if PV_LDW_DEDUP:
        strip_redundant_ldweights(nc)
    nc.compile()
    strip_redundant_ldweights(nc, strip=False)  # verify only
    return nc


def prep_core_inputs(Qc, Kc, Vc):
    """Host-side layout prep for one core's [heads, s, d] fp16 slices."""
    heads, s, d = Qc.shape
    qt = np.ascontiguousarray(Qc.transpose(0, 2, 1))
    k4 = Kc.reshape(heads, 128, s // 128, d)
    kt = np.ascontiguousarray(k4.transpose(0, 3, 2, 1)).reshape(heads, d, s)
    va = np.concatenate([Vc, np.ones((heads, s, 1), np.float16)], axis=2)
    return {"qt": qt, "kt": kt, "va": np.ascontiguousarray(va)}


_cache = {}


def kernel(Q, K, V):
    Q = np.asarray(Q, dtype=np.float16)
    K = np.asarray(K, dtype=np.float16)
    V = np.asarray(V, dtype=np.float16)
    b, h, s, d = Q.shape
    assert (b, h, s, d) == (B, H, S, D)

    if "nc" not in _cache:
        _cache["nc"] = build_program()
    nc = _cache["nc"]

    Qf = Q.reshape(b * h, s, d)
    Kf = K.reshape(b * h, s, d)
    Vf = V.reshape(b * h, s, d)
    in_maps = [
        prep_core_inputs(
            Qf[c * HPC : (c + 1) * HPC],
            Kf[c * HPC : (c + 1) * HPC],
            Vf[c * HPC : (c + 1) * HPC],
        )
        for c in range(N_CORES)
    ]
    res = run_bass_kernel_spmd(nc, in_maps, core_ids=list(range(N_CORES)))
    outs = [res.results[c]["o"] for c in range(N_CORES)]
    return np.concatenate(outs, axis=0).reshape(b, h, s, d)

